# revision 1
# baseline (speedup 1.0000x reference)
"""BiMamba forward kernel for 8 TRN2 NeuronCores.

Sharding: core c = (batch b, direction dir, d_inner half h); the host
pre-flips reverse-direction inputs in time so the device program is
identical (purely causal) on all cores. Each core produces a partial
output projection [d_model, L]; the host sums four partials per batch
element (unflipping the reverse ones). A host-side channel permutation
puts this core's d_inner half in x-path tiles 0..5 so the single SPMD
program needs no per-core branches.

Device layout: channels on partitions, time on the free dim. The scan is
hardware tensor_tensor_scan (h = dA*h + dBu along time) on VectorE, one
instruction per (128-channel tile, state s, time half). The scan stage
runs in two time halves so the 32 B/C broadcast tiles fit in SBUF and
are loaded once per half instead of once per (r,s). dbu and h bounce
through PSUM to keep their traffic off the SBUF ports, which are the
shared bottleneck across VectorE/GpSimd/ScalarE. The x_dbl projection is
computed as a half-channel partial and summed across half-pairs with a
pairwise AllReduce.
"""
import numpy as np
import ml_dtypes

import concourse.bass as bass
import concourse.tile as tile
from concourse import bacc, mybir
from concourse.bass_utils import run_bass_kernel_spmd

D_MODEL = 768
D_INNER = 1536
D_STATE = 16
D_CONV = 4
DT_RANK = 48
BATCH = 2
SEQLEN = 2048

HALF = D_INNER // 2
NDT = HALF // 128            # 6 half d-tiles
NDT_FULL = D_INNER // 128    # 12 full d-tiles
NK = D_MODEL // 128          # 6 k-tiles over d_model
L = SEQLEN
NCH = 4
CW = L // NCH                # 512
LH = L // 2                  # 1024 time half
NHC = LH // CW               # 2 chunks of 512 per half
NXD = DT_RANK + 2 * D_STATE  # 80
NXP = 96                     # x_dbl psum rows padded: B/C at partition 64
NM = D_MODEL // 128          # 6 out-proj row tiles

F32 = mybir.dt.float32
BF16 = mybir.dt.bfloat16
BF_NP = ml_dtypes.bfloat16

N_S_F32 = 0       # decay planes in fp32 for s < N_S_F32
PSUM_DBU = False  # bf16 PSUM writes are matmul-only on TRN2
PSUM_H = False
# states whose dbu / ws multiply runs on GpSimd (rest on VectorE)
POOL_DBU = frozenset()
POOL_WS = frozenset()

AF = mybir.ActivationFunctionType
OP = mybir.AluOpType


def build_program(debug_stage=0):
    nc = bacc.Bacc("TRN2", target_bir_lowering=False, debug=False,
                   num_devices=8)
    dram = {}

    def din(name, shape, dt):
        dram[name] = nc.dram_tensor(name, list(shape), dt,
                                    kind="ExternalInput").ap()

    def dout(name, shape, dt):
        dram[name] = nc.dram_tensor(name, list(shape), dt,
                                    kind="ExternalOutput").ap()

    din("uT", (D_MODEL, L), BF16)
    din("w_in_xT", (D_MODEL, HALF), BF16)
    din("w_in_zT", (D_MODEL, HALF), BF16)
    din("conv_diag", (NDT * D_CONV * 128, 128), BF16)
    din("conv_b", (HALF, 1), F32)
    din("w_xT", (HALF, NXP), BF16)
    din("w_dtT", (DT_RANK, HALF), BF16)
    din("b_dt", (HALF, 1), F32)
    din("A_half", (HALF, D_STATE), F32)
    din("dp_diag", (NDT * 128, 128), BF16)
    din("idn", (128, 128), BF16)
    din("w_outT", (HALF, D_MODEL), BF16)
    dout("out_part", (D_MODEL, L), F32)

    with tile.TileContext(nc) as tc:
        _body(nc, tc, dram)
    nc.compile()
    return nc


def _body(nc, tc, dram):
    with tc.tile_pool(name="wpool", bufs=1) as wp, \
         tc.tile_pool(name="dramp", bufs=1, space="DRAM") as dp_pool:

        # ---- DRAM scratch ----
        bc_scr = dp_pool.tile([2 * D_STATE, L], BF16, name="bc_scr")
        cc_in = [dp_pool.tile([NXP, LH], F32, name=f"cc_in{h}")
                 for h in range(2)]
        cc_out = [dp_pool.tile([NXP, LH], F32, name=f"cc_out{h}")
                  for h in range(2)]

        # ---- persistent small weights ----
        idn = wp.tile([128, 128], BF16, name="idn")
        nc.sync.dma_start(idn[:], dram["idn"][:])
        dp_diag = [wp.tile([128, 128], BF16, name=f"dpd{r}")
                   for r in range(NDT)]
        A_col = [wp.tile([128, D_STATE], F32, name=f"acol{r}")
                 for r in range(NDT)]
        b_dt = [wp.tile([128, 1], F32, name=f"bdt{r}") for r in range(NDT)]
        conv_b = [wp.tile([128, 1], F32, name=f"cvb{r}")
                  for r in range(NDT)]
        for r in range(NDT):
            nc.sync.dma_start(dp_diag[r][:],
                              dram["dp_diag"][r * 128:(r + 1) * 128, :])
            nc.sync.dma_start(A_col[r][:],
                              dram["A_half"][r * 128:(r + 1) * 128, :])
            nc.sync.dma_start(b_dt[r][:],
                              dram["b_dt"][r * 128:(r + 1) * 128, :])
            nc.sync.dma_start(conv_b[r][:],
                              dram["conv_b"][r * 128:(r + 1) * 128, :])
        w_dtT = wp.tile([DT_RANK, HALF], BF16, name="w_dtT")
        nc.sync.dma_start(w_dtT[:], dram["w_dtT"][:])
        w_outT = [wp.tile([128, D_MODEL], BF16, name=f"wout{r}")
                  for r in range(NDT)]
        for r in range(NDT):
            nc.sync.dma_start(w_outT[r][:],
                              dram["w_outT"][r * 128:(r + 1) * 128, :])
        w_xT = [wp.tile([128, NXP], BF16, name=f"wx{k}")
                for k in range(NDT)]
        for k in range(NDT):
            nc.sync.dma_start(w_xT[k][:],
                              dram["w_xT"][k * 128:(k + 1) * 128, :])

        with tc.tile_pool(name="hold", bufs=1) as hold:
            # resident activations: conv+silu x path, gated z, dt rows
            xc_bf = [hold.tile([128, L], BF16, name=f"xc{r}")
                     for r in range(NDT)]
            uT = [hold.tile([128, L], BF16, name=f"uT{k}")
                  for k in range(NK)]
            w_in_zT = [hold.tile([128, HALF], BF16, name=f"wiz{k}")
                       for k in range(NK)]
            dtT_bf = hold.tile([DT_RANK, L], BF16, name="dtT_bf")
            # per-(r,s) scan carry: h at the end of time half 0
            carry = [hold.tile([128, D_STATE], F32, name=f"carry{r}")
                     for r in range(NDT)]

            _stages_pre(nc, tc, dram, wp, locals())
            _scan_halves(nc, tc, dram, wp, locals())


def _stages_pre(nc, tc, dram, wp, env):
    """in-proj x -> conv -> silu (resident xc), x_dbl partial -> pairwise
    AllReduce (z-proj + silu overlaps the collective), cc readback."""
    xc_bf = env["xc_bf"]
    uT = env["uT"]
    w_in_zT = env["w_in_zT"]
    dtT_bf = env["dtT_bf"]
    conv_b = env["conv_b"]
    w_xT = env["w_xT"]
    bc_scr = env["bc_scr"]
    cc_in = env["cc_in"]
    cc_out = env["cc_out"]
    LPAD = L + 3

    with tc.tile_pool(name="pre", bufs=1) as p3, \
         tc.tile_pool(name="ps_a", bufs=2, space="PSUM") as psa:
        for k in range(NK):
            nc.sync.dma_start(uT[k][:],
                              dram["uT"][k * 128:(k + 1) * 128, :])
        for k in range(NK):
            nc.sync.dma_start(w_in_zT[k][:],
                              dram["w_in_zT"][k * 128:(k + 1) * 128, :])
        with tc.tile_pool(name="pre12", bufs=1) as p12:
            w_in_xT = [p12.tile([128, HALF], BF16, name=f"wix{k}",
                                tag=f"wix{k}") for k in range(NK)]
            for k in range(NK):
                nc.sync.dma_start(w_in_xT[k][:],
                                  dram["w_in_xT"][k * 128:(k + 1) * 128, :])
            conv_diag = [p12.tile([128, 128], BF16, name=f"cvd{i}",
                                  tag=f"cvd{i}")
                         for i in range(NDT * D_CONV)]
            for i in range(NDT * D_CONV):
                nc.sync.dma_start(conv_diag[i][:],
                                  dram["conv_diag"][i * 128:(i + 1) * 128, :])

            # stages 1+2 per d-tile, half-0 columns first so the half-0
            # x_dbl AllReduce launches as early as possible; xr persists
            # per r so half-1's conv sees the half-0 tail columns
            xr_t = [p12.tile([128, LPAD], BF16, name=f"xr{r}",
                             tag=f"xr{r}") for r in range(NDT)]
            for hf in range(2):
                for r in range(NDT):
                    xr = xr_t[r]
                    if hf == 0:
                        nc.vector.memset(xr[:, 0:3], 0.0)
                    for n in range(hf * NHC, (hf + 1) * NHC):
                        ps = psa.tile([128, CW], F32, name="psa",
                                      tag="psa")
                        for k in range(NK):
                            nc.tensor.matmul(
                                ps[:],
                                w_in_xT[k][:, r * 128:(r + 1) * 128],
                                uT[k][:, n * CW:(n + 1) * CW],
                                start=(k == 0), stop=(k == NK - 1))
                        nc.scalar.copy(xr[:, 3 + n * CW:3 + (n + 1) * CW],
                                       ps[:])
                    for n in range(hf * NHC, (hf + 1) * NHC):
                        ps = psa.tile([128, CW], F32, name="psa",
                                      tag="psa")
                        for j in range(D_CONV):
                            nc.tensor.matmul(
                                ps[:], conv_diag[r * D_CONV + j][:],
                                xr[:, n * CW + j:n * CW + j + CW],
                                start=(j == 0), stop=(j == D_CONV - 1))
                        nc.scalar.activation(
                            xc_bf[r][:, n * CW:(n + 1) * CW], ps[:],
                            AF.Silu, bias=conv_b[r][:], scale=1.0)
                # launch this half's x_dbl partial + AllReduce right away
                _xdbl_cc(nc, env, p3, psa, hf)

        # read back the pair-reduced half-0 x_dbl as soon as it lands
        _cc_read(nc, env, p3, 0)


def _xdbl_cc(nc, env, pool, psa, hf):
    """x_dbl partial over time half hf -> DRAM -> pairwise AllReduce."""
    xc_bf = env["xc_bf"]
    w_xT = env["w_xT"]
    cc_in = env["cc_in"]
    cc_out = env["cc_out"]
    t0 = hf * LH
    xdbl_f = pool.tile([NXP, LH], F32, name=f"xdf{hf}", tag=f"xdf{hf}")
    for n in range(NHC):
        ps = psa.tile([NXP, CW], F32, name="ps3", tag="ps3", bufs=2)
        for k in range(NDT):
            nc.tensor.matmul(
                ps[:], w_xT[k][:],
                xc_bf[k][:, t0 + n * CW:t0 + (n + 1) * CW],
                start=(k == 0), stop=(k == NDT - 1))
        nc.scalar.copy(xdbl_f[:, n * CW:(n + 1) * CW], ps[:])
    nc.sync.dma_start(cc_in[hf][:], xdbl_f[:])
    nc.gpsimd.collective_compute(
        "AllReduce", mybir.AluOpType.add,
        replica_groups=[[0, 1], [2, 3], [4, 5], [6, 7]],
        ins=[cc_in[hf][:]], outs=[cc_out[hf][:]])


def _cc_read(nc, env, pool, hf):
    """Read the pair-reduced x_dbl for half hf into dtT / bc_scr."""
    dtT_bf = env["dtT_bf"]
    bc_scr = env["bc_scr"]
    cc_out = env["cc_out"]
    t0 = hf * LH
    xdbl_r = pool.tile([NXP, LH], F32, name=f"xdr{hf}", tag=f"xdr{hf}")
    nc.sync.dma_start(xdbl_r[:], cc_out[hf][:])
    nc.scalar.copy(dtT_bf[:, t0:t0 + LH], xdbl_r[0:DT_RANK, :])
    bcb = pool.tile([2 * D_STATE, LH], BF16, name=f"bcb{hf}",
                    tag=f"bcb{hf}")
    nc.scalar.copy(bcb[:], xdbl_r[64:NXP, :])
    nc.sync.dma_start(bc_scr[:, t0:t0 + LH], bcb[:])


def _scan_halves(nc, tc, dram, wp, env):
    xc_bf = env["xc_bf"]
    uT = env["uT"]
    w_in_zT = env["w_in_zT"]
    dtT_bf = env["dtT_bf"]
    carry = env["carry"]
    bc_scr = env["bc_scr"]
    w_dtT = env["w_dtT"]
    A_col = env["A_col"]
    b_dt = env["b_dt"]
    dp_diag = env["dp_diag"]
    idn = env["idn"]
    w_outT = env["w_outT"]

    with tc.tile_pool(name="repp", bufs=1) as rp, \
         tc.tile_pool(name="scanp", bufs=1) as sp, \
         tc.tile_pool(name="ps_mm4", bufs=2, space="PSUM") as ps4, \
         tc.tile_pool(name="ps_y", bufs=1, space="PSUM") as psy, \
         tc.tile_pool(name="ps_o", bufs=2, space="PSUM") as ps6, \
         tc.tile_pool(name="op6", bufs=1) as p6:
        reps = {}

        def load_reps(hf):
            t0 = hf * LH
            b_rep = [rp.tile([128, LH], BF16, name=f"br{s}", tag=f"br{s}",
                             bufs=(2 if s < 1 else 1))
                     for s in range(D_STATE)]
            c_rep = [rp.tile([128, LH], BF16, name=f"cr{s}", tag=f"cr{s}",
                             bufs=(2 if s < 1 else 1))
                     for s in range(D_STATE)]
            for s in range(D_STATE):
                nc.sync.dma_start(
                    b_rep[s][:], bc_scr[s:s + 1, t0:t0 + LH]
                    .broadcast_to((128, LH)))
                nc.sync.dma_start(
                    c_rep[s][:],
                    bc_scr[D_STATE + s:D_STATE + s + 1, t0:t0 + LH]
                    .broadcast_to((128, LH)))
            reps[hf] = (b_rep, c_rep)

        load_reps(0)
        for hf in range(2):
            t0 = hf * LH
            b_rep, c_rep = reps[hf]

            yg_bf = [sp.tile([128, LH], BF16, name=f"yg{r}",
                             tag=f"yg{r}") for r in range(NDT)]

            for r in range(NDT):
                # ---- delta = softplus(dt @ W_dt.T + b_dt) on ACT ----
                tA = sp.tile([128, LH], BF16, name="tA", tag="tA")
                zmx = sp.tile([128, LH], BF16, name="zmx", tag="zmx")
                for n in range(NHC):
                    ps = ps4.tile([128, CW], F32, name="ps4t", tag="ps4t")
                    nc.tensor.matmul(
                        ps[:], w_dtT[:, r * 128:(r + 1) * 128],
                        dtT_bf[:, t0 + n * CW:t0 + (n + 1) * CW],
                        start=True, stop=True)
                    nc.scalar.activation(tA[:, n * CW:(n + 1) * CW],
                                         ps[:], AF.Abs, bias=b_dt[r][:],
                                         scale=1.0)
                    nc.scalar.activation(zmx[:, n * CW:(n + 1) * CW],
                                         ps[:], AF.Relu, bias=b_dt[r][:],
                                         scale=1.0)
                tB = sp.tile([128, LH], BF16, name="tB", tag="tB")
                nc.scalar.activation(tB[:], tA[:], AF.Exp, bias=0.0,
                                     scale=-1.0)              # exp(-|z|)
                tL = sp.tile([128, LH], BF16, name="tL", tag="tL")
                nc.scalar.activation(tL[:], tB[:], AF.Ln, bias=1.0,
                                     scale=1.0)               # ln(1+e)
                delta = sp.tile([128, LH], BF16, name="delta", tag="delta",
                                bufs=2)
                nc.vector.tensor_tensor(delta[:], zmx[:], tL[:], OP.add)

                # ---- du = delta * xc ----
                du = sp.tile([128, LH], BF16, name="du", tag="du", bufs=2)
                nc.vector.tensor_tensor(du[:], delta[:],
                                        xc_bf[r][:, t0:t0 + LH], OP.mult)

                yp = [psy.tile([128, CW], F32, name=f"yp{n}", tag=f"yp{n}")
                      for n in range(NHC)]

                for s in range(D_STATE):
                    if s < N_S_F32:
                        dA = sp.tile([128, LH], F32, name="ef", tag="ef",
                                     bufs=2)
                    else:
                        dA = sp.tile([128, LH], BF16, name="eb", tag="eb",
                                     bufs=3)
                    nc.scalar.activation(dA[:], delta[:], AF.Exp, bias=0.0,
                                         scale=A_col[r][:, s:s + 1])
                    if PSUM_DBU:
                        dbu = psd.tile([128, LH], BF16, name="dbu",
                                       tag="dbu")
                    else:
                        dbu = sp.tile([128, LH], BF16, name="dbu",
                                      tag="dbu", bufs=2)
                    eng = nc.gpsimd if s in POOL_DBU else nc.vector
                    eng.tensor_tensor(dbu[:], du[:], b_rep[s][:], OP.mult)
                    if PSUM_H:
                        h = psh.tile([128, LH], BF16, name="h", tag="h")
                    else:
                        h = sp.tile([128, LH], BF16, name="h", tag="h",
                                    bufs=3)
                    init = 0.0 if hf == 0 else carry[r][:, s:s + 1]
                    nc.vector.tensor_tensor_scan(h[:], dA[:], dbu[:], init,
                                                 OP.mult, OP.add)
                    if hf == 0:
                        nc.scalar.copy(carry[r][:, s:s + 1],
                                       h[:, LH - 1:LH])
                    ws = sp.tile([128, LH], BF16, name="ws", tag="ws",
                                 bufs=2)
                    eng2 = nc.gpsimd if s in POOL_WS else nc.vector
                    eng2.tensor_tensor(ws[:], h[:], c_rep[s][:], OP.mult)
                    for n in range(NHC):
                        nc.tensor.matmul(yp[n][:], idn[:],
                                         ws[:, n * CW:(n + 1) * CW],
                                         start=(s == 0), stop=False)
                # skip term
                for n in range(NHC):
                    nc.tensor.matmul(
                        yp[n][:], dp_diag[r][:],
                        xc_bf[r][:, t0 + n * CW:t0 + (n + 1) * CW],
                        start=False, stop=True)

                # z-projection + silu for this (r, half) — emitted after
                # the 16 set-6 exps so the Silu table reload stays off the
                # critical dA path
                gzt = sp.tile([128, LH], BF16, name="gzt", tag="gzt",
                              bufs=2)
                for n in range(NHC):
                    ps = ps4.tile([128, CW], F32, name="psz", tag="ps4t")
                    for k in range(NK):
                        nc.tensor.matmul(
                            ps[:], w_in_zT[k][:, r * 128:(r + 1) * 128],
                            uT[k][:, t0 + n * CW:t0 + (n + 1) * CW],
                            start=(k == 0), stop=(k == NK - 1))
                    nc.scalar.activation(gzt[:, n * CW:(n + 1) * CW],
                                         ps[:], AF.Silu)
                # gate with silu(z)
                for n in range(NHC):
                    nc.vector.tensor_tensor(
                        yg_bf[r][:, n * CW:(n + 1) * CW], yp[n][:],
                        gzt[:, n * CW:(n + 1) * CW], OP.mult)

                # pipeline half-1 readback under half-0 scans
                if hf == 0:
                    if r == 2:
                        _cc_read(nc, env, sp, 1)
                    elif r == 3:
                        load_reps(1)

            # ---- out-proj for this half ----
            for m in range(NM):
                for n in range(NHC):
                    ps = ps6.tile([128, CW], F32, name="ps6t", tag="ps6t")
                    for r in range(NDT):
                        nc.tensor.matmul(
                            ps[:], w_outT[r][:, m * 128:(m + 1) * 128],
                            yg_bf[r][:, n * CW:(n + 1) * CW],
                            start=(r == 0), stop=(r == NDT - 1))
                    ot = p6.tile([128, CW], F32, name="ot", tag="ot",
                                 bufs=2)
                    nc.scalar.copy(ot[:], ps[:])
                    nc.sync.dma_start(
                        dram["out_part"][m * 128:(m + 1) * 128,
                                         t0 + n * CW:t0 + (n + 1) * CW],
                        ot[:])


# ======================= host side =======================

def _prep_core_inputs(inputs, b, rev, h):
    hs = np.asarray(inputs["hidden_states"])
    W_in = np.asarray(inputs["W_in"])
    conv_w = np.asarray(inputs["conv_w"])[:, 0, :]
    conv_b = np.asarray(inputs["conv_b"])
    W_x = np.asarray(inputs["W_x"])
    W_dt = np.asarray(inputs["W_dt"])
    b_dt = np.asarray(inputs["b_dt"])
    A = -np.exp(np.asarray(inputs["A_log"], np.float64)).astype(np.float32)
    Dp = np.asarray(inputs["Dp"])
    W_out = np.asarray(inputs["W_out"])

    lo, hi = h * HALF, (h + 1) * HALF

    u = hs[b]
    if rev:
        u = u[::-1]
    uT = np.ascontiguousarray(u.T).astype(BF_NP)

    W_in_x = W_in[lo:hi]
    W_in_z = W_in[D_INNER + lo:D_INNER + hi]
    conv_wp = conv_w[lo:hi]
    conv_bp = conv_b[lo:hi].reshape(-1, 1).astype(np.float32)
    W_xp = W_x[:, lo:hi]
    W_xpad = np.zeros((NXP, W_xp.shape[1]), W_xp.dtype)
    W_xpad[0:DT_RANK] = W_xp[0:DT_RANK]
    W_xpad[64:96] = W_xp[DT_RANK:NXD]

    conv_diag = np.zeros((NDT * D_CONV * 128, 128), np.float32)
    idx = np.arange(128)
    for r in range(NDT):
        for j in range(D_CONV):
            base = (r * D_CONV + j) * 128
            conv_diag[base + idx, idx] = conv_wp[r * 128:(r + 1) * 128, j]

    dp_diag = np.zeros((NDT * 128, 128), np.float32)
    for r in range(NDT):
        dp_diag[r * 128 + idx, idx] = Dp[lo + r * 128: lo + (r + 1) * 128]

    return {
        "uT": uT,
        "w_in_xT": np.ascontiguousarray(W_in_x.T).astype(BF_NP),
        "w_in_zT": np.ascontiguousarray(W_in_z.T).astype(BF_NP),
        "conv_diag": conv_diag.astype(BF_NP),
        "conv_b": conv_bp,
        "w_xT": np.ascontiguousarray(W_xpad.T).astype(BF_NP),
        "w_dtT": np.ascontiguousarray(W_dt[lo:hi].T).astype(BF_NP),
        "b_dt": b_dt[lo:hi].reshape(-1, 1).astype(np.float32),
        "A_half": A[lo:hi].astype(np.float32),
        "dp_diag": dp_diag.astype(BF_NP),
        "idn": np.eye(128, dtype=np.float32).astype(BF_NP),
        "w_outT": np.ascontiguousarray(W_out[:, lo:hi].T).astype(BF_NP),
    }


_CACHE = {}


def kernel(**inputs):
    if "prog" not in _CACHE:
        _CACHE["prog"] = build_program(0)
    nc = _CACHE["prog"]

    in_maps = []
    for c in range(8):
        b, rev, h = c >> 2, (c >> 1) & 1, c & 1
        in_maps.append(_prep_core_inputs(inputs, b, rev, h))
    res = run_bass_kernel_spmd(nc, in_maps, list(range(8)))

    out = np.zeros((BATCH, L, D_MODEL), np.float32)
    for c in range(8):
        b, rev, h = c >> 2, (c >> 1) & 1, c & 1
        part = res.results[c]["out_part"].T
        if rev:
            part = part[::-1]
        out[b] += part
    return out



# revision 52
# speedup vs baseline: 9.9006x; 9.9006x over previous
"""BiMamba forward kernel for 8 TRN2 NeuronCores.

Sharding: core c = (batch b, direction dir, d_inner half h); the host
pre-flips reverse-direction inputs in time so the device program is
identical (purely causal) on all cores. Each core produces a partial
output projection [d_model, L] (bf16); the host sums four partials per
batch element (unflipping the reverse ones).

No collectives: each core computes the FULL 1536-channel x path
(in-proj + depthwise conv) so the x_dbl projection contracts locally.
The host permutes channels so this core's scan half sits in x-path
tiles 0..5; tiles 6..11 exist only to feed x_dbl.

Device layout: channels on partitions, time on free dim, two time
halves. Per (r, s): dA = exp(A_s * delta) on ScalarE; dbu multiplies on
VectorE (bf16 2x, feeding the VectorE-only tensor_tensor_scan without a
cross-engine hop); most ws multiplies on the otherwise-idle GpSimd;
state reduction via identity-matmul PSUM accumulation on PE. The y*silu
gate for tile r is deferred into tile r+1's VectorE stream so GpSimd's
trailing ws work never stalls VectorE. softplus is computed directly as
Ln(1+Exp(x)) (x = dt-proj + b_dt stays < ~6 for this model), keeping
the scan phase in the single natural_log_exp activation table; Silu
work (conv, z-gate) is batched per interleave window. Half-1's
in-proj/conv/x_dbl is interleaved under half-0's scans, finishing early
enough that the half-1 B/C broadcast DMAs stream in behind the half-0
tail instead of stalling the boundary.
"""
import numpy as np
import ml_dtypes

import concourse.bass as bass
import concourse.tile as tile
from concourse import bacc, mybir
from concourse.bass_utils import run_bass_kernel_spmd

D_MODEL = 768
D_INNER = 1536
D_STATE = 16
D_CONV = 4
DT_RANK = 48
BATCH = 2
SEQLEN = 2048

HALF = D_INNER // 2
NDT = HALF // 128            # 6 own-half d-tiles (scanned)
NDTF = D_INNER // 128        # 12 full d-tiles (x path)
NK = D_MODEL // 128          # 6 k-tiles over d_model
L = SEQLEN
LH = L // 2                  # 1024 time half
CW = 512                     # matmul free chunk
NHC = LH // CW               # 2 chunks per half
NXD = DT_RANK + 2 * D_STATE  # 80
NXP = 96                     # x_dbl rows padded: B/C at partition 64
NM = D_MODEL // 128          # 6 out-proj row tiles

F32 = mybir.dt.float32
BF16 = mybir.dt.bfloat16
BF_NP = ml_dtypes.bfloat16

# which states' dbu / ws multiplies run on GpSimd (rest on VectorE)
POOL_DBU = frozenset()
POOL_WS = frozenset(range(0, 15))

# half-1 stage-A tiles emitted after each half-0 scan tile (own tile k
# may only appear at position >= k: its xch buffer is reused); windows
# are consolidated so Silu<->Exp/Ln act-table flips stay rare
FG_SCHED = {0: [6, 7], 1: [8, 9], 2: [10, 11], 3: [0, 1],
            4: [2, 3], 5: [4, 5]}
# half-1 z-proj tiles attached to each window (Silu work); tile zr's
# gzt rewrite must follow the DEFERRED half-0 gating of zr, which is
# emitted inside scan_r(0, zr+1) — so window r may carry zr <= r-2.
ZH1_SCHED = {2: [0], 3: [1], 4: [2], 5: [3]}

AF = mybir.ActivationFunctionType
OP = mybir.AluOpType

NPRE = 4  # dA exps prefetched per tile ahead of the interleave window

def _minimize_act_loads(nc):
    """The compiler's table-load pass assigns each activation function a
    fixed canonical table (first act_info set containing it) and inserts
    a load on every canonical-table change — so streams mixing Silu with
    Copy, or Exp with Ln, reload constantly even though one resident
    table covers whole regions. Replace its loads with a minimal set:
    track the actually-resident table and switch only when the next
    function is genuinely absent, preferring natural_log_exp_and_others
    (Exp+Ln+Copy+Abs+Relu) and silu_and_others (Silu+Copy)."""
    from concourse.hw_specs import get_activation_tables
    tabs = get_activation_tables(nc.m.arch)
    names = list(tabs)
    name2id = {n: i for i, n in enumerate(names)}
    prefer = ["natural_log_exp_and_others", "silu_and_others"]

    def pick(func):
        for n in prefer:
            if func in tabs[n]:
                return n
        for n in names:
            if func in tabs[n]:
                return n
        raise ValueError(f"no act table contains {func}")

    for blk in nc.main_func.blocks:
        insts = list(blk.instructions)
        new = []
        cur = None
        for inst in insts:
            if isinstance(inst, mybir.InstLoadActFuncSet):
                si = inst.sync_info
                if si is not None and (si.on_wait or si.on_update):
                    new.append(inst)      # carries sems; keep untouched
                    cur = tabs[names[inst.act_func_set_id]]
                continue
            if (isinstance(inst, mybir.InstActivation)
                    and inst.engine == mybir.EngineType.Activation):
                f = inst.func
                if cur is None or f not in cur:
                    n = pick(f)
                    new.append(mybir.InstLoadActFuncSet(
                        name=nc.get_next_instruction_name(),
                        engine=mybir.EngineType.Activation,
                        act_func_set_id=name2id[n], ins=[], outs=[]))
                    cur = tabs[n]
            new.append(inst)
        if len(new) != len(insts):
            blk.instructions = new
    return nc


def build_program(debug_stage=0):
    nc = _build_program_inner()
    return _minimize_act_loads(nc)


def _build_program_inner():
    nc = bacc.Bacc("TRN2", target_bir_lowering=False, debug=False,
                   num_devices=8)
    dram = {}

    def din(name, shape, dt):
        dram[name] = nc.dram_tensor(name, list(shape), dt,
                                    kind="ExternalInput").ap()

    def dout(name, shape, dt):
        dram[name] = nc.dram_tensor(name, list(shape), dt,
                                    kind="ExternalOutput").ap()

    din("uT", (D_MODEL, L), BF16)
    din("w_in_xT", (D_MODEL, D_INNER), BF16)
    din("w_in_zT", (D_MODEL, HALF), BF16)
    din("conv_diag", (128, NDTF * D_CONV * 128), BF16)
    din("conv_b", (128, NDTF), F32)
    din("w_xT", (128, NDTF * NXP), BF16)
    din("w_dtT", (DT_RANK, HALF), BF16)
    din("b_dt", (128, NDT), F32)
    din("A_half", (128, NDT * D_STATE), F32)
    din("dp_diag", (128, NDT * 128), BF16)
    din("idn", (128, 128), BF16)
    din("w_outT", (HALF, D_MODEL), BF16)
    dout("out_part", (D_MODEL, L), BF16)

    with tile.TileContext(nc) as tc:
        _body(nc, tc, dram)
    nc.compile()
    return nc


def _body(nc, tc, dram):
    with tc.tile_pool(name="wpool", bufs=1) as wp, \
         tc.tile_pool(name="dramp", bufs=1, space="DRAM") as dp_pool:

        bc_scr = dp_pool.tile([2 * D_STATE, L], BF16, name="bc_scr")

        # ---- tiles (loads are emitted in _schedule, critical-path first)
        # small per-tile weights are packed side-by-side in one wide tile
        # per family so each loads with a single DMA
        idn = wp.tile([128, 128], BF16, name="idn")
        dp_flat = wp.tile([128, NDT * 128], BF16, name="dp_flat")
        dp_diag = [dp_flat[:, r * 128:(r + 1) * 128] for r in range(NDT)]
        A_flat = wp.tile([128, NDT * D_STATE], F32, name="A_flat")
        A_col = [A_flat[:, r * D_STATE:(r + 1) * D_STATE]
                 for r in range(NDT)]
        bdt_flat = wp.tile([128, NDT], F32, name="bdt_flat")
        b_dt = [bdt_flat[:, r:r + 1] for r in range(NDT)]
        cvb_flat = wp.tile([128, NDTF], F32, name="cvb_flat")
        conv_b = [cvb_flat[:, r:r + 1] for r in range(NDTF)]
        w_dtT = wp.tile([DT_RANK, HALF], BF16, name="w_dtT")
        w_outT = [wp.tile([128, D_MODEL], BF16, name=f"wout{r}")
                  for r in range(NDT)]
        wx_flat = wp.tile([128, NDTF * NXP], BF16, name="wx_flat")
        w_xT = [wx_flat[:, k * NXP:(k + 1) * NXP] for k in range(NDTF)]
        w_in_xT = [wp.tile([128, D_INNER], BF16, name=f"wix{k}")
                   for k in range(NK)]
        w_in_zT = [wp.tile([128, HALF], BF16, name=f"wiz{k}")
                   for k in range(NK)]
        cvd_flat = wp.tile([128, NDTF * D_CONV * 128], BF16,
                           name="cvd_flat")
        conv_diag = [cvd_flat[:, i * 128:(i + 1) * 128]
                     for i in range(NDTF * D_CONV)]

        carry = [wp.tile([128, D_STATE], F32, name=f"carry{r}")
                 for r in range(NDT)]
        xr_tail = [wp.tile([128, D_CONV - 1], BF16, name=f"xtl{r}")
                   for r in range(NDTF)]
        # dt rows (0:48) and B/C rows (64:96) share one staging tile in
        # the x_dbl PSUM layout
        xbT = wp.tile([NXP, L], BF16, name="xbT")

        env = dict(idn=idn, dp_diag=dp_diag, A_col=A_col, b_dt=b_dt,
                   conv_b=conv_b, w_dtT=w_dtT, w_outT=w_outT, w_xT=w_xT,
                   w_in_xT=w_in_xT, w_in_zT=w_in_zT, conv_diag=conv_diag,
                   dp_flat=dp_flat, A_flat=A_flat, bdt_flat=bdt_flat,
                   cvb_flat=cvb_flat, wx_flat=wx_flat, cvd_flat=cvd_flat,
                   carry=carry, xr_tail=xr_tail, xbT=xbT,
                   bc_scr=bc_scr, dram=dram)

        with tc.tile_pool(name="hold", bufs=1) as hold, \
             tc.tile_pool(name="bcp", bufs=1) as bcp, \
             tc.tile_pool(name="trans", bufs=1) as trans, \
             tc.tile_pool(name="ps_rot", bufs=2, space="PSUM") as ps_rot, \
             tc.tile_pool(name="ps_xd", bufs=1, space="PSUM") as ps_xd, \
             tc.tile_pool(name="ps_y", bufs=2, space="PSUM") as ps_y:
            # per-half activation tiles (tag-reused between halves)
            env["uTh"] = [hold.tile([128, LH], BF16, name=f"uTh{k}",
                                    tag=f"uTh{k}") for k in range(NK)]
            env["xch"] = [hold.tile([128, LH], BF16, name=f"xch{r}",
                                    tag=f"xch{r}") for r in range(NDT)]
            env["gzt"] = [hold.tile([128, LH], BF16, name=f"gzt{r}",
                                    tag=f"gzt{r}") for r in range(NDT)]
            env["yg"] = [hold.tile([128, LH], BF16, name=f"yg{r}",
                                   tag=f"yg{r}") for r in range(NDT)]
            env["pools"] = dict(hold=hold, bcp=bcp, trans=trans,
                                ps_rot=ps_rot, ps_xd=ps_xd, ps_y=ps_y)
            _schedule(nc, tc, env)


def _load_primary(nc, env):
    """weights needed by stage A, in use order; wix/uTh interleaved so
    the k=0 in-proj matmul can start after the first pair lands."""
    dram = env["dram"]
    for k in range(NK):
        nc.sync.dma_start(env["w_in_xT"][k][:],
                          dram["w_in_xT"][k * 128:(k + 1) * 128, :])
        nc.sync.dma_start(env["uTh"][k][:],
                          dram["uT"][k * 128:(k + 1) * 128, 0:LH])
    nc.sync.dma_start(env["cvd_flat"][:], dram["conv_diag"][:])
    nc.sync.dma_start(env["cvb_flat"][:], dram["conv_b"][:])
    nc.sync.dma_start(env["wx_flat"][:], dram["w_xT"][:])


def _load_secondary(nc, env):
    """weights for z-proj / scan / out-proj; they stream in behind the
    stage-A critical path."""
    dram = env["dram"]
    for k in range(NK):
        nc.sync.dma_start(env["w_in_zT"][k][:],
                          dram["w_in_zT"][k * 128:(k + 1) * 128, :])
    nc.sync.dma_start(env["w_dtT"][:], dram["w_dtT"][:])
    nc.sync.dma_start(env["idn"][:], dram["idn"][:])
    nc.sync.dma_start(env["A_flat"][:], dram["A_half"][:])
    nc.sync.dma_start(env["bdt_flat"][:], dram["b_dt"][:])
    nc.sync.dma_start(env["dp_flat"][:], dram["dp_diag"][:])
    for r in range(NDT):
        nc.sync.dma_start(env["w_outT"][r][:],
                          dram["w_outT"][r * 128:(r + 1) * 128, :])


def _load_uth(nc, env, hf):
    t0 = hf * LH
    for k in range(NK):
        nc.sync.dma_start(env["uTh"][k][:],
                          env["dram"]["uT"][k * 128:(k + 1) * 128,
                                            t0:t0 + LH])


def _inproj_tile(nc, env, hf, r):
    """in-proj x for tile r over half hf -> transient xr (with conv pad)."""
    trans = env["pools"]["trans"]
    ps_rot = env["pools"]["ps_rot"]
    PAD = D_CONV - 1
    xr = trans.tile([128, PAD + LH], BF16, name="xr", tag="xr", bufs=1)
    if hf == 0:
        nc.vector.memset(xr[:, 0:PAD], 0.0)
    else:
        nc.scalar.copy(xr[:, 0:PAD], env["xr_tail"][r][:])
    for n in range(NHC):
        ps = ps_rot.tile([128, CW], F32, name="psA", tag="psr")
        for k in range(NK):
            nc.tensor.matmul(
                ps[:], env["w_in_xT"][k][:, r * 128:(r + 1) * 128],
                env["uTh"][k][:, n * CW:(n + 1) * CW],
                start=(k == 0), stop=(k == NK - 1))
        nc.scalar.copy(xr[:, PAD + n * CW:PAD + (n + 1) * CW], ps[:])
    if hf == 0:
        nc.scalar.copy(env["xr_tail"][r][:], xr[:, LH:LH + PAD])
    return xr


def _conv_silu_xdbl(nc, env, hf, r, xr, xd_ps, first, last):
    """conv + silu for tile r (into xch[r] if own half else transient),
    then accumulate x_dbl. `after`: Act instruction the Silus must
    follow in the schedule (groups table flips)."""
    trans = env["pools"]["trans"]
    ps_rot = env["pools"]["ps_rot"]
    if r < NDT:
        dst = env["xch"][r]
    else:
        dst = trans.tile([128, LH], BF16, name="xco", tag="xco", bufs=1)
    for n in range(NHC):
        ps = ps_rot.tile([128, CW], F32, name="psB", tag="psr")
        for j in range(D_CONV):
            nc.tensor.matmul(ps[:], env["conv_diag"][r * D_CONV + j][:],
                             xr[:, n * CW + j:n * CW + j + CW],
                             start=(j == 0), stop=(j == D_CONV - 1))
        act = nc.scalar.activation(dst[:, n * CW:(n + 1) * CW], ps[:],
                                   AF.Silu, bias=env["conv_b"][r][:],
                                   scale=1.0)
        prev = env.get("_silu_chain")
        if prev is not None:
            bass._add_dep_helper(act.ins, prev.ins, sync=False,
                                 reason="silu window chain")
        env["_silu_chain"] = act
    for n in range(NHC):
        nc.tensor.matmul(xd_ps[n][0:NXP, :], env["w_xT"][r][:],
                         dst[:, n * CW:(n + 1) * CW],
                         start=first, stop=last)


def _extract_xdbl(nc, env, hf, xd_ps):
    """x_dbl PSUM -> staging (dt rows 0:48, B/C rows 64:96) -> DRAM."""
    t0 = hf * LH
    for n in range(NHC):
        nc.scalar.copy(env["xbT"][:, t0 + n * CW:t0 + (n + 1) * CW],
                       xd_ps[n][0:NXP, :])
    nc.sync.dma_start(env["bc_scr"][:, t0:t0 + LH],
                      env["xbT"][64:NXP, t0:t0 + LH])


def _load_bc(nc, env, hf):
    """Broadcast B rows to 128 partitions on the SP HWDGE queue; C rows
    go via the Activation HWDGE queue, dispatched later (see
    _dispatch_c) so the two queues stream in parallel without the Act
    sequencer stalling on WAR waits at the half boundary."""
    bcp = env["pools"]["bcp"]
    t0 = hf * LH
    b_rep = [bcp.tile([128, LH], BF16, name=f"br{s}", tag=f"br{s}")
             for s in range(D_STATE)]
    c_rep = [bcp.tile([128, LH], BF16, name=f"cr{s}", tag=f"cr{s}")
             for s in range(D_STATE)]
    for s in range(D_STATE):
        nc.sync.dma_start(
            b_rep[s][:],
            env["bc_scr"][s:s + 1, t0:t0 + LH].broadcast_to((128, LH)))
        nc.sync.dma_start(
            c_rep[s][:],
            env["bc_scr"][D_STATE + s:D_STATE + s + 1, t0:t0 + LH]
            .broadcast_to((128, LH)))
    return b_rep, c_rep


def _dispatch_c(nc, env, hf, c_rep):
    pass


def _z_silu(nc, env, r, borrow_xd=False):
    """z-proj + silu for own tile r over the CURRENT half's uTh. When
    the x_dbl banks are idle (z(0) batch, boundary z's) borrow them so
    the silu cadence isn't throttled by the shared rotating PSUM tag."""
    pool = env["pools"]["ps_xd" if borrow_xd else "ps_rot"]
    for n in range(NHC):
        ps = (pool.tile([128, CW], F32, name="psZ", tag=f"xd{n}")
              if borrow_xd else
              pool.tile([128, CW], F32, name="psZ", tag="psr"))
        for k in range(NK):
            nc.tensor.matmul(
                ps[:], env["w_in_zT"][k][:, r * 128:(r + 1) * 128],
                env["uTh"][k][:, n * CW:(n + 1) * CW],
                start=(k == 0), stop=(k == NK - 1))
        act = nc.scalar.activation(env["gzt"][r][:, n * CW:(n + 1) * CW],
                                   ps[:], AF.Silu)
        prev = env.get("_silu_chain")
        if prev is not None:
            bass._add_dep_helper(act.ins, prev.ins, sync=False,
                                 reason="silu window chain")
        env["_silu_chain"] = act
    return act


def _scan_head(nc, env, hf, r, after_act=None):
    """delta / du / first NPRE dA exps for tile r — emitted BEFORE the
    previous tile's interleave window so VectorE stays fed while the
    window's Silu work occupies ScalarE."""
    trans = env["pools"]["trans"]
    ps_rot = env["pools"]["ps_rot"]
    t0 = hf * LH

    # delta = softplus(dt @ W_dt.T + b_dt) = Ln(1 + Exp(x)); x < ~6 here
    eT = trans.tile([128, LH], BF16, name="eT", tag="eT", bufs=2)
    for n in range(NHC):
        ps = ps_rot.tile([128, CW], F32, name="psD", tag="psr")
        nc.tensor.matmul(ps[:],
                         env["w_dtT"][:, r * 128:(r + 1) * 128],
                         env["xbT"][0:DT_RANK,
                                    t0 + n * CW:t0 + (n + 1) * CW],
                         start=True, stop=True)
        act = nc.scalar.activation(eT[:, n * CW:(n + 1) * CW], ps[:],
                                   AF.Exp, bias=env["b_dt"][r][:],
                                   scale=1.0)
        if after_act is not None:
            bass._add_dep_helper(act.ins, after_act.ins, sync=False,
                                 reason="act stream order")
            after_act = None
    # delta shares eT's slots (bufs=2): delta(r) lands in the buffer the
    # previous r's delta occupied; eT(r+1) reuses this r's eT slot.
    delta = trans.tile([128, LH], BF16, name="delta", tag="eT", bufs=2)
    nc.scalar.activation(delta[:], eT[:], AF.Ln, bias=1.0, scale=1.0)

    du = trans.tile([128, LH], BF16, name="du", tag="du", bufs=2)
    nc.vector.tensor_tensor(du[:], delta[:], env["xch"][r][:], OP.mult)

    dAs = {}
    last = None
    for s_i in range(NPRE):
        dA = trans.tile([128, LH], BF16, name="dA", tag="dA", bufs=NPRE)
        last = nc.scalar.activation(dA[:], delta[:], AF.Exp, bias=0.0,
                                    scale=env["A_col"][r][:, s_i:s_i + 1])
        dAs[s_i] = dA
    return dict(delta=delta, du=du, dAs=dAs, last_act=last)


def _scan_body(nc, env, hf, r, head, b_rep, c_rep, prev_gate,
               after_act=None):
    """s-loop for tile r; returns (deferred gating closure, last Act
    instruction) — window Silu ops are ordered after that instruction."""
    trans = env["pools"]["trans"]
    ps_y = env["pools"]["ps_y"]
    delta, du, dAs = head["delta"], head["du"], head["dAs"]
    last_act = None

    yp = [ps_y.tile([128, CW], F32, name=f"yp{n}", tag=f"yp{n}")
          for n in range(NHC)]

    for s in range(D_STATE):
        if s in dAs:
            dA = dAs[s]
        else:
            dA = trans.tile([128, LH], BF16, name="dA", tag="dA",
                            bufs=NPRE)
            last_act = nc.scalar.activation(
                dA[:], delta[:], AF.Exp, bias=0.0,
                scale=env["A_col"][r][:, s:s + 1])
            if after_act is not None:
                bass._add_dep_helper(last_act.ins, after_act.ins,
                                     sync=False,
                                     reason="act after silu window")
                after_act = None
        dbu = trans.tile([128, LH], BF16, name="dbu", tag="dbu", bufs=2)
        eng = nc.gpsimd if s in POOL_DBU else nc.vector
        eng.tensor_tensor(dbu[:], du[:], b_rep[s][:], OP.mult)
        h = trans.tile([128, LH], BF16, name="h", tag="h", bufs=3)
        init = 0.0 if hf == 0 else env["carry"][r][:, s:s + 1]
        nc.vector.tensor_tensor_scan(h[:], dA[:], dbu[:], init,
                                     OP.mult, OP.add)
        if hf == 0:
            nc.scalar.copy(env["carry"][r][:, s:s + 1], h[:, LH - 1:LH])
        ws = trans.tile([128, LH], BF16, name="ws", tag="ws", bufs=2)
        eng2 = nc.gpsimd if s in POOL_WS else nc.vector
        eng2.tensor_tensor(ws[:], h[:], c_rep[s][:], OP.mult)
        for n in range(NHC):
            nc.tensor.matmul(yp[n][:], env["idn"][:],
                             ws[:, n * CW:(n + 1) * CW],
                             start=(s == 0), stop=False)
        if s == 2 and prev_gate is not None:
            prev_gate()
            prev_gate = None
    # skip term D * xc
    for n in range(NHC):
        nc.tensor.matmul(yp[n][:], env["dp_diag"][r][:],
                         env["xch"][r][:, n * CW:(n + 1) * CW],
                         start=False, stop=True)

    def gate():
        for n in range(NHC):
            nc.vector.tensor_tensor(
                env["yg"][r][:, n * CW:(n + 1) * CW], yp[n][:],
                env["gzt"][r][:, n * CW:(n + 1) * CW], OP.mult)
    return gate, last_act


def _out_proj(nc, env, hf):
    trans = env["pools"]["trans"]
    ps_rot = env["pools"]["ps_rot"]
    t0 = hf * LH
    for m in range(NM):
        for n in range(NHC):
            ps = ps_rot.tile([128, CW], F32, name="psO", tag="psr")
            for r in range(NDT):
                nc.tensor.matmul(
                    ps[:], env["w_outT"][r][:, m * 128:(m + 1) * 128],
                    env["yg"][r][:, n * CW:(n + 1) * CW],
                    start=(r == 0), stop=(r == NDT - 1))
            ot = trans.tile([128, CW], BF16, name="ot", tag="ot", bufs=2)
            nc.scalar.copy(ot[:], ps[:])
            nc.sync.dma_start(
                env["dram"]["out_part"][m * 128:(m + 1) * 128,
                                        t0 + n * CW:t0 + (n + 1) * CW],
                ot[:])


def _schedule(nc, tc, env):
    ps_xd = env["pools"]["ps_xd"]

    # ---- half-0 lead: full x path for all 12 tiles ----
    _load_primary(nc, env)
    _load_secondary(nc, env)
    xd0 = [ps_xd.tile([128, CW], F32, name=f"xd{n}", tag=f"xd{n}")
           for n in range(NHC)]
    env["_silu_chain"] = None
    for r in range(NDTF):
        xr = _inproj_tile(nc, env, 0, r)
        _conv_silu_xdbl(nc, env, 0, r, xr, xd0,
                        first=(r == 0), last=(r == NDTF - 1))
    _extract_xdbl(nc, env, 0, xd0)
    b0, c0 = _load_bc(nc, env, 0)
    # delta(0,0) first so VectorE ramps while the z(0) Silu batch runs
    head = _scan_head(nc, env, 0, 0)
    _dispatch_c(nc, env, 0, c0)
    env["_silu_chain"] = head["last_act"]
    zlast = None
    for r in range(NDT):
        zlast = _z_silu(nc, env, r)
    # prefetch half-1 u while half-0 scans run (uTh(0) fully consumed:
    # stage-A in-proj and all six z(0) projections are emitted above)
    _load_uth(nc, env, 1)

    # ---- half-0 scans with interleaved half-1 stage A ----
    xd1 = [ps_xd.tile([128, CW], F32, name=f"xd{n}", tag=f"xd{n}")
           for n in range(NHC)]
    gate = None
    pending = zlast
    head1 = None
    for r in range(NDT):
        gate, last_act = _scan_body(nc, env, 0, r, head, b0, c0, gate,
                                    after_act=pending)
        pending = None
        if r + 1 < NDT:
            head = _scan_head(nc, env, 0, r + 1)
        if FG_SCHED.get(r):
            # window silus: contiguous Act block after the next head's
            # exps; the following body's exps are ordered after them
            env["_silu_chain"] = (head["last_act"] if r + 1 < NDT
                                  else last_act)
            for t in FG_SCHED[r]:
                xr = _inproj_tile(nc, env, 1, t)
                _conv_silu_xdbl(nc, env, 1, t, xr, xd1,
                                first=(t == 6), last=(t == NDT - 1))
            for zr in ZH1_SCHED.get(r, []):
                _z_silu(nc, env, zr)
            pending = env["_silu_chain"]
    _extract_xdbl(nc, env, 1, xd1)
    b1, c1 = _load_bc(nc, env, 1)
    gate()
    # half-1 z for tiles 4,5: need gating(0,4)/(0,5) emitted (above)
    env["_silu_chain"] = None
    _z_silu(nc, env, 4)
    zlast = _z_silu(nc, env, 5)

    # ---- half-1 scans (half-0 out-proj slotted into r0's slack) ----
    gate = None
    head = _scan_head(nc, env, 1, 0, after_act=zlast)
    for r in range(NDT):
        gate, _ = _scan_body(nc, env, 1, r, head, b1, c1, gate)
        if r + 1 < NDT:
            head = _scan_head(nc, env, 1, r + 1)
        if r == 0:
            _out_proj(nc, env, 0)
    gate()
    _out_proj(nc, env, 1)


# ======================= host side =======================

def _prep_core_inputs(inputs, b, rev, h):
    hs = np.asarray(inputs["hidden_states"])
    W_in = np.asarray(inputs["W_in"])
    conv_w = np.asarray(inputs["conv_w"])[:, 0, :]
    conv_b = np.asarray(inputs["conv_b"])
    W_x = np.asarray(inputs["W_x"])
    W_dt = np.asarray(inputs["W_dt"])
    b_dt = np.asarray(inputs["b_dt"])
    A = -np.exp(np.asarray(inputs["A_log"], np.float64)).astype(np.float32)
    Dp = np.asarray(inputs["Dp"])
    W_out = np.asarray(inputs["W_out"])

    lo, hi = h * HALF, (h + 1) * HALF
    olo, ohi = (1 - h) * HALF, (2 - h) * HALF
    perm = np.concatenate([np.arange(lo, hi), np.arange(olo, ohi)])

    u = hs[b]
    if rev:
        u = u[::-1]
    uT = np.ascontiguousarray(u.T).astype(BF_NP)

    W_in_x = W_in[0:D_INNER][perm]          # (1536, 768) permuted
    W_in_z = W_in[D_INNER + lo:D_INNER + hi]
    conv_wp = conv_w[perm]                  # (1536, 4)
    conv_bp = conv_b[perm].reshape(-1, 1).astype(np.float32)
    W_xp = W_x[:, perm]                     # (80, 1536)
    W_xpad = np.zeros((NXP, D_INNER), W_xp.dtype)
    W_xpad[0:DT_RANK] = W_xp[0:DT_RANK]
    W_xpad[64:96] = W_xp[DT_RANK:NXD]

    idx = np.arange(128)
    conv_diag = np.zeros((128, NDTF * D_CONV * 128), np.float32)
    for r in range(NDTF):
        for j in range(D_CONV):
            base = (r * D_CONV + j) * 128
            conv_diag[idx, base + idx] = conv_wp[r * 128:(r + 1) * 128, j]

    cvb_flat = np.zeros((128, NDTF), np.float32)
    for r in range(NDTF):
        cvb_flat[:, r] = conv_bp[r * 128:(r + 1) * 128, 0]

    wx_flat = np.zeros((128, NDTF * NXP), np.float32)
    W_xpT = W_xpad.T                        # (1536, 96)
    for k in range(NDTF):
        wx_flat[:, k * NXP:(k + 1) * NXP] = W_xpT[k * 128:(k + 1) * 128]

    bdt_flat = np.zeros((128, NDT), np.float32)
    A_flat = np.zeros((128, NDT * D_STATE), np.float32)
    dp_flat = np.zeros((128, NDT * 128), np.float32)
    for r in range(NDT):
        bdt_flat[:, r] = b_dt[lo + r * 128:lo + (r + 1) * 128]
        A_flat[:, r * D_STATE:(r + 1) * D_STATE] = \
            A[lo + r * 128:lo + (r + 1) * 128]
        dp_flat[idx, r * 128 + idx] = Dp[lo + r * 128:lo + (r + 1) * 128]

    return {
        "uT": uT,
        "w_in_xT": np.ascontiguousarray(W_in_x.T).astype(BF_NP),
        "w_in_zT": np.ascontiguousarray(W_in_z.T).astype(BF_NP),
        "conv_diag": conv_diag.astype(BF_NP),
        "conv_b": cvb_flat,
        "w_xT": wx_flat.astype(BF_NP),
        "w_dtT": np.ascontiguousarray(W_dt[lo:hi].T).astype(BF_NP),
        "b_dt": bdt_flat,
        "A_half": A_flat,
        "dp_diag": dp_flat.astype(BF_NP),
        "idn": np.eye(128, dtype=np.float32).astype(BF_NP),
        "w_outT": np.ascontiguousarray(W_out[:, lo:hi].T).astype(BF_NP),
    }


_CACHE = {}


def kernel(**inputs):
    if "prog" not in _CACHE:
        _CACHE["prog"] = build_program(0)
    nc = _CACHE["prog"]

    in_maps = []
    for c in range(8):
        b, rev, h = c >> 2, (c >> 1) & 1, c & 1
        in_maps.append(_prep_core_inputs(inputs, b, rev, h))
    res = run_bass_kernel_spmd(nc, in_maps, list(range(8)))

    out = np.zeros((BATCH, L, D_MODEL), np.float32)
    for c in range(8):
        b, rev, h = c >> 2, (c >> 1) & 1, c & 1
        part = res.results[c]["out_part"].astype(np.float32).T
        if rev:
            part = part[::-1]
        out[b] += part
    return out


# revision 57
# speedup vs baseline: 9.9021x; 1.0001x over previous
"""BiMamba forward kernel for 8 TRN2 NeuronCores.

Sharding: core c = (batch b, direction dir, d_inner half h); the host
pre-flips reverse-direction inputs in time so the device program is
identical (purely causal) on all cores. Each core produces a partial
output projection [d_model, L] (bf16); the host sums four partials per
batch element (unflipping the reverse ones).

No collectives: each core computes the FULL 1536-channel x path
(in-proj + depthwise conv) so the x_dbl projection contracts locally.
The host permutes channels so this core's scan half sits in x-path
tiles 0..5; tiles 6..11 exist only to feed x_dbl.

Device layout: channels on partitions, time on free dim, two time
halves. Per (r, s): dA = exp(A_s * delta) on ScalarE; dbu multiplies on
VectorE (bf16 2x, feeding the VectorE-only tensor_tensor_scan without a
cross-engine hop); most ws multiplies on the otherwise-idle GpSimd;
state reduction via identity-matmul PSUM accumulation on PE. The y*silu
gate for tile r is deferred into tile r+1's VectorE stream so GpSimd's
trailing ws work never stalls VectorE. softplus is computed directly as
Ln(1+Exp(x)) (x = dt-proj + b_dt stays < ~6 for this model), keeping
the scan phase in the single natural_log_exp activation table; Silu
work (conv, z-gate) is batched per interleave window. Half-1's
in-proj/conv/x_dbl is interleaved under half-0's scans, finishing early
enough that the half-1 B/C broadcast DMAs stream in behind the half-0
tail instead of stalling the boundary.
"""
import numpy as np
import ml_dtypes

import concourse.bass as bass
import concourse.tile as tile
from concourse import bacc, mybir
from concourse.bass_utils import run_bass_kernel_spmd

D_MODEL = 768
D_INNER = 1536
D_STATE = 16
D_CONV = 4
DT_RANK = 48
BATCH = 2
SEQLEN = 2048

HALF = D_INNER // 2
NDT = HALF // 128            # 6 own-half d-tiles (scanned)
NDTF = D_INNER // 128        # 12 full d-tiles (x path)
NK = D_MODEL // 128          # 6 k-tiles over d_model
L = SEQLEN
LH = L // 2                  # 1024 time half
CW = 512                     # matmul free chunk
NHC = LH // CW               # 2 chunks per half
NXD = DT_RANK + 2 * D_STATE  # 80
NXP = 96                     # x_dbl rows padded: B/C at partition 64
NM = D_MODEL // 128          # 6 out-proj row tiles

F32 = mybir.dt.float32
BF16 = mybir.dt.bfloat16
BF_NP = ml_dtypes.bfloat16

# which states' dbu / ws multiplies run on GpSimd (rest on VectorE)
POOL_DBU = frozenset()
POOL_WS = frozenset(range(0, 15))

# half-1 stage-A tiles emitted after each half-0 scan tile (own tile k
# may only appear at position >= k: its xch buffer is reused); windows
# are consolidated so Silu<->Exp/Ln act-table flips stay rare
FG_SCHED = {0: [6, 7], 1: [8, 9], 2: [10, 11], 3: [0, 1],
            4: [2, 3], 5: [4, 5]}
# half-1 z-proj tiles attached to each window (Silu work); tile zr's
# gzt rewrite must follow the DEFERRED half-0 gating of zr, which is
# emitted inside scan_r(0, zr+1) — so window r may carry zr <= r-2.
ZH1_SCHED = {2: [0], 3: [1], 4: [2], 5: [3]}

AF = mybir.ActivationFunctionType
OP = mybir.AluOpType

NPRE = 4  # dA exps prefetched per tile ahead of the interleave window

def _minimize_act_loads(nc):
    """The compiler's table-load pass assigns each activation function a
    fixed canonical table (first act_info set containing it) and inserts
    a load on every canonical-table change — so streams mixing Silu with
    Copy, or Exp with Ln, reload constantly even though one resident
    table covers whole regions. Replace its loads with a minimal set:
    track the actually-resident table and switch only when the next
    function is genuinely absent, preferring natural_log_exp_and_others
    (Exp+Ln+Copy+Abs+Relu) and silu_and_others (Silu+Copy)."""
    from concourse.hw_specs import get_activation_tables
    tabs = get_activation_tables(nc.m.arch)
    names = list(tabs)
    name2id = {n: i for i, n in enumerate(names)}
    prefer = ["natural_log_exp_and_others", "silu_and_others"]

    def pick(func):
        for n in prefer:
            if func in tabs[n]:
                return n
        for n in names:
            if func in tabs[n]:
                return n
        raise ValueError(f"no act table contains {func}")

    for blk in nc.main_func.blocks:
        insts = list(blk.instructions)
        new = []
        cur = None
        for inst in insts:
            if isinstance(inst, mybir.InstLoadActFuncSet):
                si = inst.sync_info
                if si is not None and (si.on_wait or si.on_update):
                    new.append(inst)      # carries sems; keep untouched
                    cur = tabs[names[inst.act_func_set_id]]
                continue
            if (isinstance(inst, mybir.InstActivation)
                    and inst.engine == mybir.EngineType.Activation):
                f = inst.func
                if cur is None or f not in cur:
                    n = pick(f)
                    new.append(mybir.InstLoadActFuncSet(
                        name=nc.get_next_instruction_name(),
                        engine=mybir.EngineType.Activation,
                        act_func_set_id=name2id[n], ins=[], outs=[]))
                    cur = tabs[n]
            new.append(inst)
        if len(new) != len(insts):
            blk.instructions = new
    return nc


def build_program(debug_stage=0):
    nc = _build_program_inner()
    return _minimize_act_loads(nc)


def _build_program_inner():
    nc = bacc.Bacc("TRN2", target_bir_lowering=False, debug=False,
                   num_devices=8)
    dram = {}

    def din(name, shape, dt):
        dram[name] = nc.dram_tensor(name, list(shape), dt,
                                    kind="ExternalInput").ap()

    def dout(name, shape, dt):
        dram[name] = nc.dram_tensor(name, list(shape), dt,
                                    kind="ExternalOutput").ap()

    din("uT", (D_MODEL, L), BF16)
    din("w_in_xT", (D_MODEL, D_INNER), BF16)
    din("w_in_zT", (D_MODEL, HALF), BF16)
    din("conv_diag", (128, NDTF * D_CONV * 128), BF16)
    din("conv_b", (128, NDTF), F32)
    din("w_xT", (128, NDTF * NXP), BF16)
    din("w_dtT", (DT_RANK, HALF), BF16)
    din("b_dt", (128, NDT), F32)
    din("A_half", (128, NDT * D_STATE), F32)
    din("dp_diag", (128, NDT * 128), BF16)
    din("idn", (128, 128), BF16)
    din("w_outT", (HALF, D_MODEL), BF16)
    dout("out_part", (D_MODEL, L), BF16)

    with tile.TileContext(nc) as tc:
        _body(nc, tc, dram)
    nc.compile()
    return nc


def _body(nc, tc, dram):
    with tc.tile_pool(name="wpool", bufs=1) as wp, \
         tc.tile_pool(name="dramp", bufs=1, space="DRAM") as dp_pool:

        bc_scr = dp_pool.tile([2 * D_STATE, L], BF16, name="bc_scr")

        # ---- tiles (loads are emitted in _schedule, critical-path first)
        # small per-tile weights are packed side-by-side in one wide tile
        # per family so each loads with a single DMA
        idn = wp.tile([128, 128], BF16, name="idn")
        dp_flat = wp.tile([128, NDT * 128], BF16, name="dp_flat")
        dp_diag = [dp_flat[:, r * 128:(r + 1) * 128] for r in range(NDT)]
        A_flat = wp.tile([128, NDT * D_STATE], F32, name="A_flat")
        A_col = [A_flat[:, r * D_STATE:(r + 1) * D_STATE]
                 for r in range(NDT)]
        bdt_flat = wp.tile([128, NDT], F32, name="bdt_flat")
        b_dt = [bdt_flat[:, r:r + 1] for r in range(NDT)]
        cvb_flat = wp.tile([128, NDTF], F32, name="cvb_flat")
        conv_b = [cvb_flat[:, r:r + 1] for r in range(NDTF)]
        w_dtT = wp.tile([DT_RANK, HALF], BF16, name="w_dtT")
        w_outT = [wp.tile([128, D_MODEL], BF16, name=f"wout{r}")
                  for r in range(NDT)]
        wx_flat = wp.tile([128, NDTF * NXP], BF16, name="wx_flat")
        w_xT = [wx_flat[:, k * NXP:(k + 1) * NXP] for k in range(NDTF)]
        w_in_xT = [wp.tile([128, D_INNER], BF16, name=f"wix{k}")
                   for k in range(NK)]
        w_in_zT = [wp.tile([128, HALF], BF16, name=f"wiz{k}")
                   for k in range(NK)]
        cvd_flat = wp.tile([128, NDTF * D_CONV * 128], BF16,
                           name="cvd_flat")
        conv_diag = [cvd_flat[:, i * 128:(i + 1) * 128]
                     for i in range(NDTF * D_CONV)]

        carry = [wp.tile([128, D_STATE], F32, name=f"carry{r}")
                 for r in range(NDT)]
        xr_tail = [wp.tile([128, D_CONV - 1], BF16, name=f"xtl{r}")
                   for r in range(NDTF)]
        # dt rows (0:48) and B/C rows (64:96) share one staging tile in
        # the x_dbl PSUM layout
        xbT = wp.tile([NXP, L], BF16, name="xbT")

        env = dict(idn=idn, dp_diag=dp_diag, A_col=A_col, b_dt=b_dt,
                   conv_b=conv_b, w_dtT=w_dtT, w_outT=w_outT, w_xT=w_xT,
                   w_in_xT=w_in_xT, w_in_zT=w_in_zT, conv_diag=conv_diag,
                   dp_flat=dp_flat, A_flat=A_flat, bdt_flat=bdt_flat,
                   cvb_flat=cvb_flat, wx_flat=wx_flat, cvd_flat=cvd_flat,
                   carry=carry, xr_tail=xr_tail, xbT=xbT,
                   bc_scr=bc_scr, dram=dram)

        with tc.tile_pool(name="hold", bufs=1) as hold, \
             tc.tile_pool(name="bcp", bufs=1) as bcp, \
             tc.tile_pool(name="trans", bufs=1) as trans, \
             tc.tile_pool(name="ps_rot", bufs=2, space="PSUM") as ps_rot, \
             tc.tile_pool(name="ps_xd", bufs=1, space="PSUM") as ps_xd, \
             tc.tile_pool(name="ps_y", bufs=2, space="PSUM") as ps_y:
            # per-half activation tiles (tag-reused between halves)
            env["uTh"] = [hold.tile([128, LH], BF16, name=f"uTh{k}",
                                    tag=f"uTh{k}") for k in range(NK)]
            env["xch"] = [hold.tile([128, LH], BF16, name=f"xch{r}",
                                    tag=f"xch{r}") for r in range(NDT)]
            env["gzt"] = [hold.tile([128, LH], BF16, name=f"gzt{r}",
                                    tag=f"gzt{r}") for r in range(NDT)]
            env["yg"] = [hold.tile([128, LH], BF16, name=f"yg{r}",
                                   tag=f"yg{r}") for r in range(NDT)]
            env["pools"] = dict(hold=hold, bcp=bcp, trans=trans,
                                ps_rot=ps_rot, ps_xd=ps_xd, ps_y=ps_y)
            _schedule(nc, tc, env)


def _load_primary(nc, env):
    """weights needed by stage A, in use order; wix/uTh interleaved so
    the k=0 in-proj matmul can start after the first pair lands."""
    dram = env["dram"]
    for k in range(NK):
        nc.sync.dma_start(env["w_in_xT"][k][:],
                          dram["w_in_xT"][k * 128:(k + 1) * 128, :])
        nc.sync.dma_start(env["uTh"][k][:],
                          dram["uT"][k * 128:(k + 1) * 128, 0:LH])
    nc.sync.dma_start(env["cvd_flat"][:], dram["conv_diag"][:])
    nc.sync.dma_start(env["cvb_flat"][:], dram["conv_b"][:])
    nc.sync.dma_start(env["wx_flat"][:], dram["w_xT"][:])


def _load_secondary(nc, env):
    """weights for z-proj / scan / out-proj; they stream in behind the
    stage-A critical path."""
    dram = env["dram"]
    for k in range(NK):
        nc.sync.dma_start(env["w_in_zT"][k][:],
                          dram["w_in_zT"][k * 128:(k + 1) * 128, :])
    nc.sync.dma_start(env["w_dtT"][:], dram["w_dtT"][:])
    nc.sync.dma_start(env["idn"][:], dram["idn"][:])
    nc.sync.dma_start(env["A_flat"][:], dram["A_half"][:])
    nc.sync.dma_start(env["bdt_flat"][:], dram["b_dt"][:])
    nc.sync.dma_start(env["dp_flat"][:], dram["dp_diag"][:])
    for r in range(NDT):
        nc.sync.dma_start(env["w_outT"][r][:],
                          dram["w_outT"][r * 128:(r + 1) * 128, :])


def _load_uth(nc, env, hf):
    t0 = hf * LH
    for k in range(NK):
        nc.sync.dma_start(env["uTh"][k][:],
                          env["dram"]["uT"][k * 128:(k + 1) * 128,
                                            t0:t0 + LH])


def _inproj_tile(nc, env, hf, r):
    """in-proj x for tile r over half hf -> transient xr (with conv pad)."""
    trans = env["pools"]["trans"]
    ps_rot = env["pools"]["ps_rot"]
    PAD = D_CONV - 1
    xr = trans.tile([128, PAD + LH], BF16, name="xr", tag="xr", bufs=1)
    if hf == 0:
        nc.vector.memset(xr[:, 0:PAD], 0.0)
    else:
        nc.scalar.copy(xr[:, 0:PAD], env["xr_tail"][r][:])
    for n in range(NHC):
        ps = ps_rot.tile([128, CW], F32, name="psA", tag="psr")
        for k in range(NK):
            nc.tensor.matmul(
                ps[:], env["w_in_xT"][k][:, r * 128:(r + 1) * 128],
                env["uTh"][k][:, n * CW:(n + 1) * CW],
                start=(k == 0), stop=(k == NK - 1))
        nc.scalar.copy(xr[:, PAD + n * CW:PAD + (n + 1) * CW], ps[:])
    if hf == 0:
        nc.scalar.copy(env["xr_tail"][r][:], xr[:, LH:LH + PAD])
    return xr


def _conv_silu_xdbl(nc, env, hf, r, xr, xd_ps, first, last):
    """conv + silu for tile r (into xch[r] if own half else transient),
    then accumulate x_dbl. `after`: Act instruction the Silus must
    follow in the schedule (groups table flips)."""
    trans = env["pools"]["trans"]
    ps_rot = env["pools"]["ps_rot"]
    if r < NDT:
        dst = env["xch"][r]
    else:
        dst = trans.tile([128, LH], BF16, name="xco", tag="xco", bufs=1)
    for n in range(NHC):
        ps = ps_rot.tile([128, CW], F32, name="psB", tag="psr")
        for j in range(D_CONV):
            nc.tensor.matmul(ps[:], env["conv_diag"][r * D_CONV + j][:],
                             xr[:, n * CW + j:n * CW + j + CW],
                             start=(j == 0), stop=(j == D_CONV - 1))
        act = nc.scalar.activation(dst[:, n * CW:(n + 1) * CW], ps[:],
                                   AF.Silu, bias=env["conv_b"][r][:],
                                   scale=1.0)
        prev = env.get("_silu_chain")
        if prev is not None:
            bass._add_dep_helper(act.ins, prev.ins, sync=False,
                                 reason="silu window chain")
        env["_silu_chain"] = act
    for n in range(NHC):
        nc.tensor.matmul(xd_ps[n][0:NXP, :], env["w_xT"][r][:],
                         dst[:, n * CW:(n + 1) * CW],
                         start=first, stop=last)


def _extract_xdbl(nc, env, hf, xd_ps):
    """x_dbl PSUM -> staging (dt rows 0:48, B/C rows 64:96) -> DRAM."""
    t0 = hf * LH
    for n in range(NHC):
        nc.scalar.copy(env["xbT"][:, t0 + n * CW:t0 + (n + 1) * CW],
                       xd_ps[n][0:NXP, :])
    nc.sync.dma_start(env["bc_scr"][:, t0:t0 + LH],
                      env["xbT"][64:NXP, t0:t0 + LH])


def _load_bc(nc, env, hf):
    """Broadcast B rows to 128 partitions on the SP HWDGE queue; C rows
    go via the Activation HWDGE queue, dispatched later (see
    _dispatch_c) so the two queues stream in parallel without the Act
    sequencer stalling on WAR waits at the half boundary."""
    bcp = env["pools"]["bcp"]
    t0 = hf * LH
    b_rep = [bcp.tile([128, LH], BF16, name=f"br{s}", tag=f"br{s}")
             for s in range(D_STATE)]
    c_rep = [bcp.tile([128, LH], BF16, name=f"cr{s}", tag=f"cr{s}")
             for s in range(D_STATE)]
    for s in range(D_STATE):
        nc.sync.dma_start(
            b_rep[s][:],
            env["bc_scr"][s:s + 1, t0:t0 + LH].broadcast_to((128, LH)))
        nc.sync.dma_start(
            c_rep[s][:],
            env["bc_scr"][D_STATE + s:D_STATE + s + 1, t0:t0 + LH]
            .broadcast_to((128, LH)))
    return b_rep, c_rep


def _dispatch_c(nc, env, hf, c_rep):
    pass


def _z_silu(nc, env, r, borrow_xd=False):
    """z-proj + silu for own tile r over the CURRENT half's uTh. When
    the x_dbl banks are idle (z(0) batch, boundary z's) borrow them so
    the silu cadence isn't throttled by the shared rotating PSUM tag."""
    pool = env["pools"]["ps_xd" if borrow_xd else "ps_rot"]
    for n in range(NHC):
        ps = (pool.tile([128, CW], F32, name="psZ", tag=f"xd{n}")
              if borrow_xd else
              pool.tile([128, CW], F32, name="psZ", tag="psr"))
        for k in range(NK):
            nc.tensor.matmul(
                ps[:], env["w_in_zT"][k][:, r * 128:(r + 1) * 128],
                env["uTh"][k][:, n * CW:(n + 1) * CW],
                start=(k == 0), stop=(k == NK - 1))
        act = nc.scalar.activation(env["gzt"][r][:, n * CW:(n + 1) * CW],
                                   ps[:], AF.Silu)
        prev = env.get("_silu_chain")
        if prev is not None:
            bass._add_dep_helper(act.ins, prev.ins, sync=False,
                                 reason="silu window chain")
        env["_silu_chain"] = act
    return act


def _scan_head(nc, env, hf, r, after_act=None):
    """delta / du / first NPRE dA exps for tile r — emitted BEFORE the
    previous tile's interleave window so VectorE stays fed while the
    window's Silu work occupies ScalarE."""
    trans = env["pools"]["trans"]
    ps_rot = env["pools"]["ps_rot"]
    t0 = hf * LH

    # delta = softplus(dt @ W_dt.T + b_dt) = Ln(1 + Exp(x)); x < ~6 here
    eT = trans.tile([128, LH], BF16, name="eT", tag="eT", bufs=2)
    for n in range(NHC):
        ps = ps_rot.tile([128, CW], F32, name="psD", tag="psr")
        nc.tensor.matmul(ps[:],
                         env["w_dtT"][:, r * 128:(r + 1) * 128],
                         env["xbT"][0:DT_RANK,
                                    t0 + n * CW:t0 + (n + 1) * CW],
                         start=True, stop=True)
        act = nc.scalar.activation(eT[:, n * CW:(n + 1) * CW], ps[:],
                                   AF.Exp, bias=env["b_dt"][r][:],
                                   scale=1.0)
        if after_act is not None:
            bass._add_dep_helper(act.ins, after_act.ins, sync=False,
                                 reason="act stream order")
            after_act = None
    # delta shares eT's slots (bufs=2): delta(r) lands in the buffer the
    # previous r's delta occupied; eT(r+1) reuses this r's eT slot.
    delta = trans.tile([128, LH], BF16, name="delta", tag="eT", bufs=2)
    nc.scalar.activation(delta[:], eT[:], AF.Ln, bias=1.0, scale=1.0)

    du = trans.tile([128, LH], BF16, name="du", tag="du", bufs=2)
    nc.vector.tensor_tensor(du[:], delta[:], env["xch"][r][:], OP.mult)

    dAs = {}
    last = None
    for s_i in range(NPRE):
        dA = trans.tile([128, LH], BF16, name="dA", tag="dA", bufs=NPRE)
        last = nc.scalar.activation(dA[:], delta[:], AF.Exp, bias=0.0,
                                    scale=env["A_col"][r][:, s_i:s_i + 1])
        dAs[s_i] = dA
    return dict(delta=delta, du=du, dAs=dAs, last_act=last)


def _scan_body(nc, env, hf, r, head, b_rep, c_rep, prev_gate,
               after_act=None):
    """s-loop for tile r; returns (deferred gating closure, last Act
    instruction) — window Silu ops are ordered after that instruction."""
    trans = env["pools"]["trans"]
    ps_y = env["pools"]["ps_y"]
    delta, du, dAs = head["delta"], head["du"], head["dAs"]
    last_act = None

    yp = [ps_y.tile([128, CW], F32, name=f"yp{n}", tag=f"yp{n}")
          for n in range(NHC)]

    for s in range(D_STATE):
        if s in dAs:
            dA = dAs[s]
        else:
            dA = trans.tile([128, LH], BF16, name="dA", tag="dA",
                            bufs=NPRE)
            last_act = nc.scalar.activation(
                dA[:], delta[:], AF.Exp, bias=0.0,
                scale=env["A_col"][r][:, s:s + 1])
            if after_act is not None:
                bass._add_dep_helper(last_act.ins, after_act.ins,
                                     sync=False,
                                     reason="act after silu window")
                after_act = None
        dbu = trans.tile([128, LH], BF16, name="dbu", tag="dbu", bufs=2)
        eng = nc.gpsimd if s in POOL_DBU else nc.vector
        eng.tensor_tensor(dbu[:], du[:], b_rep[s][:], OP.mult)
        h = trans.tile([128, LH], BF16, name="h", tag="h", bufs=3)
        init = 0.0 if hf == 0 else env["carry"][r][:, s:s + 1]
        nc.vector.tensor_tensor_scan(h[:], dA[:], dbu[:], init,
                                     OP.mult, OP.add)
        if hf == 0:
            nc.scalar.copy(env["carry"][r][:, s:s + 1], h[:, LH - 1:LH])
        ws = trans.tile([128, LH], BF16, name="ws", tag="ws", bufs=2)
        eng2 = nc.gpsimd if s in POOL_WS else nc.vector
        eng2.tensor_tensor(ws[:], h[:], c_rep[s][:], OP.mult)
        for n in range(NHC):
            nc.tensor.matmul(yp[n][:], env["idn"][:],
                             ws[:, n * CW:(n + 1) * CW],
                             start=(s == 0), stop=False)
        if s == 2 and prev_gate is not None:
            prev_gate()
            prev_gate = None
    # skip term D * xc
    for n in range(NHC):
        nc.tensor.matmul(yp[n][:], env["dp_diag"][r][:],
                         env["xch"][r][:, n * CW:(n + 1) * CW],
                         start=False, stop=True)

    def gate():
        for n in range(NHC):
            nc.vector.tensor_tensor(
                env["yg"][r][:, n * CW:(n + 1) * CW], yp[n][:],
                env["gzt"][r][:, n * CW:(n + 1) * CW], OP.mult)
    return gate, last_act


def _out_proj(nc, env, hf):
    trans = env["pools"]["trans"]
    ps_rot = env["pools"]["ps_rot"]
    t0 = hf * LH
    for n in range(NHC):
        for m in range(NM):
            ps = ps_rot.tile([128, CW], F32, name="psO", tag="psr")
            for r in range(NDT):
                nc.tensor.matmul(
                    ps[:], env["w_outT"][r][:, m * 128:(m + 1) * 128],
                    env["yg"][r][:, n * CW:(n + 1) * CW],
                    start=(r == 0), stop=(r == NDT - 1))
            ot = trans.tile([128, CW], BF16, name="ot", tag="ot", bufs=2)
            nc.scalar.copy(ot[:], ps[:])
            nc.sync.dma_start(
                env["dram"]["out_part"][m * 128:(m + 1) * 128,
                                        t0 + n * CW:t0 + (n + 1) * CW],
                ot[:])


def _schedule(nc, tc, env):
    ps_xd = env["pools"]["ps_xd"]

    # ---- half-0 lead: full x path for all 12 tiles ----
    _load_primary(nc, env)
    _load_secondary(nc, env)
    xd0 = [ps_xd.tile([128, CW], F32, name=f"xd{n}", tag=f"xd{n}")
           for n in range(NHC)]
    env["_silu_chain"] = None
    for r in range(NDTF):
        xr = _inproj_tile(nc, env, 0, r)
        _conv_silu_xdbl(nc, env, 0, r, xr, xd0,
                        first=(r == 0), last=(r == NDTF - 1))
    _extract_xdbl(nc, env, 0, xd0)
    b0, c0 = _load_bc(nc, env, 0)
    # delta(0,0) first so VectorE ramps while the z(0) Silu batch runs
    head = _scan_head(nc, env, 0, 0)
    _dispatch_c(nc, env, 0, c0)
    env["_silu_chain"] = head["last_act"]
    zlast = None
    for r in range(NDT):
        zlast = _z_silu(nc, env, r)
    # prefetch half-1 u while half-0 scans run (uTh(0) fully consumed:
    # stage-A in-proj and all six z(0) projections are emitted above)
    _load_uth(nc, env, 1)

    # ---- half-0 scans with interleaved half-1 stage A ----
    xd1 = [ps_xd.tile([128, CW], F32, name=f"xd{n}", tag=f"xd{n}")
           for n in range(NHC)]
    gate = None
    pending = zlast
    head1 = None
    for r in range(NDT):
        gate, last_act = _scan_body(nc, env, 0, r, head, b0, c0, gate,
                                    after_act=pending)
        pending = None
        if r + 1 < NDT:
            head = _scan_head(nc, env, 0, r + 1)
        if FG_SCHED.get(r):
            # window silus: contiguous Act block after the next head's
            # exps; the following body's exps are ordered after them
            env["_silu_chain"] = (head["last_act"] if r + 1 < NDT
                                  else last_act)
            for t in FG_SCHED[r]:
                xr = _inproj_tile(nc, env, 1, t)
                _conv_silu_xdbl(nc, env, 1, t, xr, xd1,
                                first=(t == 6), last=(t == NDT - 1))
            for zr in ZH1_SCHED.get(r, []):
                _z_silu(nc, env, zr)
            pending = env["_silu_chain"]
    _extract_xdbl(nc, env, 1, xd1)
    b1, c1 = _load_bc(nc, env, 1)
    gate()
    # half-1 z for tiles 4,5: need gating(0,4)/(0,5) emitted (above)
    env["_silu_chain"] = None
    _z_silu(nc, env, 4)
    zlast = _z_silu(nc, env, 5)

    # ---- half-1 scans (half-0 out-proj slotted into r0's slack) ----
    gate = None
    head = _scan_head(nc, env, 1, 0, after_act=zlast)
    for r in range(NDT):
        gate, _ = _scan_body(nc, env, 1, r, head, b1, c1, gate)
        if r + 1 < NDT:
            head = _scan_head(nc, env, 1, r + 1)
        if r == 0:
            _out_proj(nc, env, 0)
    gate()
    _out_proj(nc, env, 1)


# ======================= host side =======================

def _prep_core_inputs(inputs, b, rev, h):
    hs = np.asarray(inputs["hidden_states"])
    W_in = np.asarray(inputs["W_in"])
    conv_w = np.asarray(inputs["conv_w"])[:, 0, :]
    conv_b = np.asarray(inputs["conv_b"])
    W_x = np.asarray(inputs["W_x"])
    W_dt = np.asarray(inputs["W_dt"])
    b_dt = np.asarray(inputs["b_dt"])
    A = -np.exp(np.asarray(inputs["A_log"], np.float64)).astype(np.float32)
    Dp = np.asarray(inputs["Dp"])
    W_out = np.asarray(inputs["W_out"])

    lo, hi = h * HALF, (h + 1) * HALF
    olo, ohi = (1 - h) * HALF, (2 - h) * HALF
    perm = np.concatenate([np.arange(lo, hi), np.arange(olo, ohi)])

    u = hs[b]
    if rev:
        u = u[::-1]
    uT = np.ascontiguousarray(u.T).astype(BF_NP)

    W_in_x = W_in[0:D_INNER][perm]          # (1536, 768) permuted
    W_in_z = W_in[D_INNER + lo:D_INNER + hi]
    conv_wp = conv_w[perm]                  # (1536, 4)
    conv_bp = conv_b[perm].reshape(-1, 1).astype(np.float32)
    W_xp = W_x[:, perm]                     # (80, 1536)
    W_xpad = np.zeros((NXP, D_INNER), W_xp.dtype)
    W_xpad[0:DT_RANK] = W_xp[0:DT_RANK]
    W_xpad[64:96] = W_xp[DT_RANK:NXD]

    idx = np.arange(128)
    conv_diag = np.zeros((128, NDTF * D_CONV * 128), np.float32)
    for r in range(NDTF):
        for j in range(D_CONV):
            base = (r * D_CONV + j) * 128
            conv_diag[idx, base + idx] = conv_wp[r * 128:(r + 1) * 128, j]

    cvb_flat = np.zeros((128, NDTF), np.float32)
    for r in range(NDTF):
        cvb_flat[:, r] = conv_bp[r * 128:(r + 1) * 128, 0]

    wx_flat = np.zeros((128, NDTF * NXP), np.float32)
    W_xpT = W_xpad.T                        # (1536, 96)
    for k in range(NDTF):
        wx_flat[:, k * NXP:(k + 1) * NXP] = W_xpT[k * 128:(k + 1) * 128]

    bdt_flat = np.zeros((128, NDT), np.float32)
    A_flat = np.zeros((128, NDT * D_STATE), np.float32)
    dp_flat = np.zeros((128, NDT * 128), np.float32)
    for r in range(NDT):
        bdt_flat[:, r] = b_dt[lo + r * 128:lo + (r + 1) * 128]
        A_flat[:, r * D_STATE:(r + 1) * D_STATE] = \
            A[lo + r * 128:lo + (r + 1) * 128]
        dp_flat[idx, r * 128 + idx] = Dp[lo + r * 128:lo + (r + 1) * 128]

    return {
        "uT": uT,
        "w_in_xT": np.ascontiguousarray(W_in_x.T).astype(BF_NP),
        "w_in_zT": np.ascontiguousarray(W_in_z.T).astype(BF_NP),
        "conv_diag": conv_diag.astype(BF_NP),
        "conv_b": cvb_flat,
        "w_xT": wx_flat.astype(BF_NP),
        "w_dtT": np.ascontiguousarray(W_dt[lo:hi].T).astype(BF_NP),
        "b_dt": bdt_flat,
        "A_half": A_flat,
        "dp_diag": dp_flat.astype(BF_NP),
        "idn": np.eye(128, dtype=np.float32).astype(BF_NP),
        "w_outT": np.ascontiguousarray(W_out[:, lo:hi].T).astype(BF_NP),
    }


_CACHE = {}


def kernel(**inputs):
    if "prog" not in _CACHE:
        _CACHE["prog"] = build_program(0)
    nc = _CACHE["prog"]

    in_maps = []
    for c in range(8):
        b, rev, h = c >> 2, (c >> 1) & 1, c & 1
        in_maps.append(_prep_core_inputs(inputs, b, rev, h))
    res = run_bass_kernel_spmd(nc, in_maps, list(range(8)))

    out = np.zeros((BATCH, L, D_MODEL), np.float32)
    for c in range(8):
        b, rev, h = c >> 2, (c >> 1) & 1, c & 1
        part = res.results[c]["out_part"].astype(np.float32).T
        if rev:
            part = part[::-1]
        out[b] += part
    return out
